# revision 1
# baseline (speedup 1.0000x reference)
"""Trainium2 Bass kernel for nn_Decoder (2-layer bidirectional-style LSTM
decoder + vocab projection), SPMD across 8 NeuronCores.

Strategy:
  - Host: embedding gather (sharding prep), weight repacking/transposition.
  - Device, per core (identical program, per-core data):
    * Gih0 = x_all @ wih_l0^T + b0, sharded over the 4096-wide gate axis
      (512 gates/core), AllGather -> every core has full Gih0.
    * Layer-0 recurrence (both directions), replicated over cores, batch on
      partitions, whh streamed as the moving operand.  h transposed via
      DMA-transpose; h^T stored to DRAM (x1^T) for the Gih1 precompute.
    * Gih1 from x1^T (gate-sharded + AllGather), layer-1 recurrence likewise,
      h1^T stored to DRAM (outs^T).
    * FC: logits chunk = outs @ fc_w[vslice]^T, vocab sharded 8 ways
      (4000 vocab rows/core).
  - Host: concat vocab slices, (s,b)->(b,s) reorder, add fc_b.
"""
import json
import os
import sys

sys.path.insert(0, "/opt/trn_rl_repo")

import ml_dtypes
import numpy as np

import concourse.bass as bass
import concourse.tile as tile
from concourse import mybir
from concourse.bass_utils import run_bass_kernel_spmd

BF16 = ml_dtypes.bfloat16
V, E, H, B, S = 32000, 512, 512, 64, 64
R = S * B              # 4096 rows, s-major: r = 64*s + b
NC = 8
VS = V // NC           # 4000 vocab rows per core
G2 = 8 * H             # 4096 = both cells' gates per layer
GB = G2 // NC          # 512 gates per core (AG mode)
USE_AG = os.environ.get("BASS_NO_AG", "") == ""
NB = 1 if USE_AG else 8          # gate blocks computed locally
GW = 512 * NB                    # local gih width

F32 = mybir.dt.float32
BF = mybir.dt.bfloat16


# --------------------------------------------------------------------------
# walrus workaround: this build allows at most 2 sem waits per instruction.
def _split_excess_waits(bir_json):
    j = json.loads(bir_json)
    n = 0
    for fn in j.get("functions", []):
        for blk in fn.get("blocks", []):
            out = []
            for inst in blk.get("instructions", []):
                si = inst.get("sync_info")
                ow = (si or {}).get("on_wait") or []
                keep = 2 if inst.get("opcode") == "EventSemaphore" else 1
                if len(ow) > keep:
                    extra, rest = ow[:-keep], ow[-keep:]
                    for i in range(0, len(extra), 2):
                        n += 1
                        out.append({
                            "debug": inst.get("debug", 0),
                            "engine": inst["engine"],
                            "ins": [], "outs": [],
                            "name": f"WSPLIT-{n}",
                            "opcode": "EventSemaphore",
                            "sync_info": {"on_update": [],
                                          "on_wait": extra[i:i + 2]},
                        })
                    si["on_wait"] = rest
                out.append(inst)
            blk["instructions"] = out
    return json.dumps(j).encode()


def _install_shim():
    import concourse.bass2jax as b2j
    import concourse.bass_utils as bu
    if getattr(bu, "_wsplit_installed", False):
        return
    orig = bu.compile_bir_kernel

    def patched(bir_json, tmpdir, neff_name="file.neff"):
        return orig(_split_excess_waits(bir_json), tmpdir, neff_name)

    bu.compile_bir_kernel = patched
    bu._wsplit_installed = True
    b2j.compile_bir_kernel = patched


# --------------------------------------------------------------------------
def build_nc():
    nc = bass.Bass()

    xfullT = nc.dram_tensor("xfullT", [128, 5, R], BF, kind="ExternalInput")
    wih0T = nc.dram_tensor("wih0T", [128, 5, GW], BF, kind="ExternalInput")
    wih1T = nc.dram_tensor("wih1T", [128, 9, GW], BF, kind="ExternalInput")
    whh0T = nc.dram_tensor("whh0T", [128, 4, G2], BF, kind="ExternalInput")
    whh1T = nc.dram_tensor("whh1T", [128, 4, G2], BF, kind="ExternalInput")
    fcwT = nc.dram_tensor("fcwT", [128, 8, VS], BF, kind="ExternalInput")
    hT0 = nc.dram_tensor("hT0", [128, 16, 64], BF, kind="ExternalInput")
    c0_in = nc.dram_tensor("c0", [64, 4, H], F32, kind="ExternalInput")
    eye_in = nc.dram_tensor("eye64", [64, 64], BF, kind="ExternalInput")
    ones_in = nc.dram_tensor("ones1", [128, R], BF, kind="ExternalInput")
    out = nc.dram_tensor("out", [R, VS], F32, kind="ExternalOutput")

    # h^T accumulators in DRAM: x1T[kc 0..7]=h0^T, chunk 8 = ones row (bias)
    x1T_d = nc.dram_tensor("x1T_d", [128, 9, R], BF)
    outsT_d = nc.dram_tensor("outsT_d", [128, 8, R], BF)
    gih_loc = [nc.dram_tensor(f"gih{l}_loc", [R, GW], BF) for l in (0, 1)]
    if USE_AG:
        gih_all = [
            nc.dram_tensor(f"gih{l}_all", [NC * R, GW], BF, addr_space="Shared")
            for l in (0, 1)
        ]

    with tile.TileContext(nc) as tc:
        with tc.tile_pool(name="persist", bufs=1) as persist:
            eye = persist.tile([64, 64], BF)
            nc.sync.dma_start(eye[:], eye_in[:])
            hTi = persist.tile([128, 16, 64], BF)
            nc.sync.dma_start(hTi[:], hT0[:])
            c_st = persist.tile([64, 4, H], F32)
            nc.sync.dma_start(c_st[:], c0_in[:])
            # ones row for x1T bias chunk (sent via DMA from input)
            nc.sync.dma_start(x1T_d[:, 8, :], ones_in[:])

            # ---- phase G0: Gih0 (gate-sharded), from host-gathered x ----
            with (
                tc.tile_pool(name="g0w", bufs=1) as g0w,
                tc.tile_pool(name="g0s", bufs=3) as g0s,
                tc.tile_pool(name="g0p", bufs=3, space="PSUM") as g0p,
            ):
                xT = g0w.tile([128, 5, R], BF)
                nc.sync.dma_start(xT[:], xfullT[:])
                w0 = g0w.tile([128, 5, GW], BF)
                nc.sync.dma_start(w0[:], wih0T[:])
                for m in range(32):
                    for nb in range(NB):
                        ps = g0p.tile([128, 512], F32)
                        for kc in range(5):
                            nc.tensor.matmul(
                                ps[:],
                                xT[:, kc, 128 * m:128 * (m + 1)],
                                w0[:, kc, 512 * nb:512 * (nb + 1)],
                                start=(kc == 0), stop=(kc == 4),
                            )
                        sb = g0s.tile([128, 512], BF)
                        nc.vector.tensor_copy(sb[:], ps[:])
                        nc.sync.dma_start(
                            gih_loc[0][128 * m:128 * (m + 1),
                                       512 * nb:512 * (nb + 1)], sb[:])
            if USE_AG:
                nc.gpsimd.collective_compute(
                    "AllGather", mybir.AluOpType.bypass,
                    ins=[gih_loc[0][:]], outs=[gih_all[0][:]],
                    replica_groups=[list(range(NC))],
                )

            # ---- recurrence (shared for both layers) ----
            def recurrence(layer, whh_sb, dstT):
                gl = layer
                with (
                    tc.tile_pool(name=f"rec{layer}_g", bufs=2) as gp_,
                    tc.tile_pool(name=f"rec{layer}_e", bufs=2) as ep,
                    tc.tile_pool(name=f"rec{layer}_h", bufs=2) as hp,
                    tc.tile_pool(name=f"rec{layer}_p", bufs=2,
                                 space="PSUM") as pp,
                ):
                    prev_hT = [None, None]
                    for s in range(S):
                        gihs = gp_.tile([64, NC, 512], BF, tag="gih")
                        if USE_AG:
                            src_v = gih_all[gl][:].rearrange(
                                "(rb r) c -> r rb c", rb=NC)
                            nc.scalar.dma_start(
                                gihs[:], src_v[64 * s:64 * (s + 1)])
                        else:
                            nc.scalar.dma_start(
                                gihs[:].rearrange("p rb c -> p (rb c)"),
                                gih_loc[gl][64 * s:64 * (s + 1), :])
                        for c in range(2):
                            cell = 2 * layer + c
                            AF = mybir.ActivationFunctionType
                            ew = {}
                            for n in range(4):
                                psn = pp.tile([64, 512], F32, tag=f"ps{n}")
                                nc.tensor.matmul(
                                    psn[:],
                                    eye[:],
                                    gihs[0:64, 4 * c + n, :],
                                    start=True, stop=False,
                                )
                                for k in range(4):
                                    if s == 0:
                                        lhsT = hTi[:, 8 * layer + 4 * c + k, :]
                                    else:
                                        lhsT = prev_hT[c][:, k, :]
                                    nc.tensor.matmul(
                                        psn[:],
                                        lhsT,
                                        whh_sb[:, k, 2048 * c + 512 * n:
                                               2048 * c + 512 * (n + 1)],
                                        start=False, stop=(k == 3),
                                    )
                                # gate nonlinearity as soon as slice n lands
                                func = AF.Tanh if n == 2 else AF.Sigmoid
                                g_sb = ep.tile([64, 512], F32, tag=f"g{n}")
                                nc.scalar.activation(psn := g_sb[:], psn[:], func) if False else \
                                    nc.scalar.activation(g_sb[:], psn[:], func)
                                ew[n] = g_sb
                            si, sf, tg, so = ew[0], ew[1], ew[2], ew[3]
                            t1 = ep.tile([64, 512], F32, tag="t1")
                            nc.vector.tensor_mul(
                                t1[:], sf[:], c_st[:, cell, :])
                            t2 = ep.tile([64, 512], F32, tag="t2")
                            nc.vector.tensor_mul(t2[:], si[:], tg[:])
                            nc.vector.tensor_add(c_st[:, cell, :],
                                                 t1[:], t2[:])
                            tc2 = ep.tile([64, 512], F32, tag="tc2")
                            nc.scalar.activation(
                                tc2[:], c_st[:, cell, :],
                                mybir.ActivationFunctionType.Tanh)
                            hn = ep.tile([64, 512], BF, tag="hn")
                            nc.vector.tensor_mul(hn[:], so[:], tc2[:])
                            # h^T via DMA transpose (ring for next step)
                            hT = hp.tile([128, 4, 64], BF, tag=f"hT{c}")
                            nc.sync.dma_start_transpose(hT[:], hn[:])
                            prev_hT[c] = hT
                            # store h^T to DRAM (Gih1 / FC stationary)
                            nc.sync.dma_start(
                                dstT[:, 4 * c:4 * (c + 1),
                                     64 * s:64 * (s + 1)], hT[:])

            # ---- layer 0 ----
            with tc.tile_pool(name="whh0", bufs=1) as wp0:
                whh0 = wp0.tile([128, 4, G2], BF)
                nc.sync.dma_start(whh0[:], whh0T[:])
                recurrence(0, whh0, x1T_d)

            # ---- phase G1: Gih1 from x1T (streamed from DRAM) ----
            with (
                tc.tile_pool(name="g1w", bufs=1) as g1w,
                tc.tile_pool(name="g1x", bufs=3) as g1x,
                tc.tile_pool(name="g1s", bufs=3) as g1s,
                tc.tile_pool(name="g1p", bufs=3, space="PSUM") as g1p,
            ):
                w1 = g1w.tile([128, 9, GW], BF)
                nc.sync.dma_start(w1[:], wih1T[:])
                for m in range(32):
                    x1m = g1x.tile([128, 9, 128], BF, tag="x1m")
                    nc.sync.dma_start(
                        x1m[:], x1T_d[:, :, 128 * m:128 * (m + 1)])
                    for nb in range(NB):
                        ps = g1p.tile([128, 512], F32)
                        for kc in range(9):
                            nc.tensor.matmul(
                                ps[:],
                                x1m[:, kc, :],
                                w1[:, kc, 512 * nb:512 * (nb + 1)],
                                start=(kc == 0), stop=(kc == 8),
                            )
                        sb = g1s.tile([128, 512], BF)
                        nc.vector.tensor_copy(sb[:], ps[:])
                        nc.sync.dma_start(
                            gih_loc[1][128 * m:128 * (m + 1),
                                       512 * nb:512 * (nb + 1)], sb[:])
            if USE_AG:
                nc.gpsimd.collective_compute(
                    "AllGather", mybir.AluOpType.bypass,
                    ins=[gih_loc[1][:]], outs=[gih_all[1][:]],
                    replica_groups=[list(range(NC))],
                )

            # ---- layer 1 ----
            with tc.tile_pool(name="whh1", bufs=1) as wp1:
                whh1 = wp1.tile([128, 4, G2], BF)
                nc.sync.dma_start(whh1[:], whh1T[:])
                recurrence(1, whh1, outsT_d)

            # ---- FC ----
            with (
                tc.tile_pool(name="fcw", bufs=1) as fwp,
                tc.tile_pool(name="fcx", bufs=3) as fxp,
                tc.tile_pool(name="fco", bufs=2) as fop,
                tc.tile_pool(name="fcp", bufs=4, space="PSUM") as fpp,
            ):
                fcw = fwp.tile([128, 8, VS], BF)
                nc.sync.dma_start(fcw[:], fcwT[:])
                for m in range(32):
                    om = fxp.tile([128, 8, 128], BF, tag="om")
                    nc.sync.dma_start(
                        om[:], outsT_d[:, :, 128 * m:128 * (m + 1)])
                    ob = fop.tile([128, VS], F32, tag="ob")
                    for n in range(8):
                        ps = fpp.tile([128, 500], F32)
                        for k in range(8):
                            nc.tensor.matmul(
                                ps[:],
                                om[:, k, :],
                                fcw[:, k, 500 * n:500 * (n + 1)],
                                start=(k == 0), stop=(k == 7),
                            )
                        nc.vector.tensor_copy(ob[:, 500 * n:500 * (n + 1)],
                                              ps[:])
                    nc.sync.dma_start(out[128 * m:128 * (m + 1), :], ob[:])
    return nc


_NC_CACHE = None


def _pack_inputs(hidden_state, cell_state, Y, emb, w_ih_l0, w_hh_l0, b_ih_l0,
                 b_hh_l0, w_ih_l1, w_hh_l1, b_ih_l1, b_hh_l1, fc_w, fc_b):
    idx_seq = np.concatenate([Y[:, 1:2], Y[:, :-1]], axis=1)  # (B,S)
    idx_flat = idx_seq.T.reshape(-1).astype(np.int64)          # r = 64s + b
    x_all = np.asarray(emb, np.float32)[idx_flat]              # (R, E)

    def packT(w, kchunks, extra_row=None):
        gdim, kk = w.shape
        kc_data = kk // 128
        outp = np.zeros((128, kchunks, gdim), BF16)
        for kc in range(kc_data):
            outp[:, kc, :] = w[:, 128 * kc:128 * (kc + 1)].T.astype(BF16)
        if extra_row is not None:
            outp[0, kc_data, :] = extra_row.astype(BF16)
        return outp

    b0 = b_ih_l0 + b_hh_l0
    b1 = b_ih_l1 + b_hh_l1
    wih0_cat = np.vstack([w_ih_l0[0], w_ih_l0[1]]).astype(np.float32)
    wih1_cat = np.vstack([w_ih_l1[0], w_ih_l1[1]]).astype(np.float32)
    whh0_cat = np.vstack([w_hh_l0[0], w_hh_l0[1]]).astype(np.float32)
    whh1_cat = np.vstack([w_hh_l1[0], w_hh_l1[1]]).astype(np.float32)
    b0_cat = np.concatenate([b0[0], b0[1]]).astype(np.float32)
    b1_cat = np.concatenate([b1[0], b1[1]]).astype(np.float32)

    xfullT = np.zeros((128, 5, R), BF16)
    for kc in range(4):
        xfullT[:, kc, :] = x_all[:, 128 * kc:128 * (kc + 1)].T.astype(BF16)
    xfullT[0, 4, :] = BF16(1.0)

    whh0T = packT(whh0_cat, 4)
    whh1T = packT(whh1_cat, 4)

    hT0 = np.zeros((128, 16, 64), BF16)
    hs = np.asarray(hidden_state, np.float32)
    for cell in range(4):
        for k in range(4):
            hT0[:, 4 * cell + k, :] = \
                hs[cell][:, 128 * k:128 * (k + 1)].T.astype(BF16)
    c0 = np.ascontiguousarray(
        np.transpose(np.asarray(cell_state, np.float32), (1, 0, 2)))
    eye64 = np.eye(64, dtype=np.float32).astype(BF16)
    ones1 = np.zeros((128, R), BF16)
    ones1[0, :] = BF16(1.0)

    fc_w = np.asarray(fc_w, np.float32)
    ins = []
    for j in range(NC):
        if USE_AG:
            gsl = slice(GB * j, GB * (j + 1))
            wih0T_j = packT(wih0_cat[gsl], 5, b0_cat[gsl])
            wih1T_j = packT(wih1_cat[gsl], 9, b1_cat[gsl])
        else:
            wih0T_j = packT(wih0_cat, 5, b0_cat)
            wih1T_j = packT(wih1_cat, 9, b1_cat)
        fcs = fc_w[VS * j:VS * (j + 1)]           # (4000, 1024)
        fcwT_j = np.zeros((128, 8, VS), BF16)
        for k in range(8):
            fcwT_j[:, k, :] = fcs[:, 128 * k:128 * (k + 1)].T.astype(BF16)
        ins.append({
            "xfullT": xfullT, "wih0T": wih0T_j, "wih1T": wih1T_j,
            "whh0T": whh0T, "whh1T": whh1T, "fcwT": fcwT_j,
            "hT0": hT0, "c0": c0, "eye64": eye64, "ones1": ones1,
        })
    return ins


def kernel(**inputs):
    global _NC_CACHE
    _install_shim()
    if _NC_CACHE is None:
        _NC_CACHE = build_nc()
    nc = _NC_CACHE
    in_maps = _pack_inputs(**inputs)
    res = run_bass_kernel_spmd(nc, in_maps, list(range(NC)))
    parts = [np.asarray(res.results[j]["out"]) for j in range(NC)]
    logits = np.concatenate(parts, axis=1)          # (R, V), r = 64s+b
    logits = logits.reshape(S, B, V).transpose(1, 0, 2).reshape(B * S, V)
    logits = logits + np.asarray(inputs["fc_b"], np.float32)[None, :]
    return logits.astype(np.float32)



# revision 22
# speedup vs baseline: 3.3576x; 3.3576x over previous
"""Trainium2 Bass kernel for nn_Decoder (2-layer bidirectional-structure LSTM
decoder + vocab projection), SPMD across 8 NeuronCores.

Strategy (v2): pure batch sharding, 8 batch rows per core, zero collectives.
All state is kept TRANSPOSED ([dim on partitions, batch on free]) so the
recurrence needs no per-step transposes:

  - gih0T = wih0^T-stationary matmuls over xT  (gates on partitions, rows on
    free), kept in SBUF.
  - L0 recurrence: per step, per gate-type t: one eye-matmul injects the gih
    slice for both cells into PSUM [128, 8(c,j), 8], then 2c*4j*4k hh-matmuls
    (N=8) accumulate h @ whh^T.  Gates -> sigmoid/tanh on ACT (one op per
    gate type, both cells merged), c/h updates on DVE.  h lands directly in
    x1T (transposed) which feeds both the next step's matmuls and gih1.
  - gih1T from x1T, L1 recurrence likewise -> outsT.
  - FC: fp8e4m3 DoubleRow matmuls (fc_w^T stationary, outsT8 moving),
    full 32000 vocab per core, logitsT written back vocab-chunk-major bf16.
  - Host: embedding gather, weight transposes/packing (shared across cores),
    final (vocab,row)->(row,vocab) reorder, fc_b add.
"""
import contextlib
import json
import os
import sys

sys.path.insert(0, "/opt/trn_rl_repo")

import ml_dtypes
import numpy as np

import concourse.bass as bass
import concourse.tile as tile
from concourse import mybir
from concourse.bass_utils import run_bass_kernel_spmd

BF16 = ml_dtypes.bfloat16
FP8 = ml_dtypes.float8_e4m3
V, E, H, B, S = 32000, 512, 512, 64, 64
NC = 8
BL = B // NC              # 8 batch rows per core
R = S * BL                # 512 rows per core, r = 8*s + bl
NV = V // 128             # 250 vocab chunks

F32 = mybir.dt.float32
BF = mybir.dt.bfloat16
F8 = mybir.dt.float8e4
AF = mybir.ActivationFunctionType

USE_FP8_FC = os.environ.get("BASS_FC_BF16", "") == ""

# gih / whh gate-chunk order: chunk = t*8 + c*4 + j
#   t: gate type in PyTorch order (i=0, f=1, g=2, o=3)
#   c: cell within layer (0=fwd, 1=bwd), j: h-dim chunk (128 wide)
# elementwise tiles are [128, 8(c,j slot=4c+j), 8(batch)]


# --------------------------------------------------------------------------
# walrus workaround: this build allows at most 2 sem waits per instruction.
def _split_excess_waits(bir_json):
    j = json.loads(bir_json)
    n = 0
    for fn in j.get("functions", []):
        for blk in fn.get("blocks", []):
            out = []
            for inst in blk.get("instructions", []):
                si = inst.get("sync_info")
                ow = (si or {}).get("on_wait") or []
                keep = 2 if inst.get("opcode") == "EventSemaphore" else 1
                if len(ow) > keep:
                    extra, rest = ow[:-keep], ow[-keep:]
                    for i in range(0, len(extra), 2):
                        n += 1
                        out.append({
                            "debug": inst.get("debug", 0),
                            "engine": inst["engine"],
                            "ins": [], "outs": [],
                            "name": f"WSPLIT-{n}",
                            "opcode": "EventSemaphore",
                            "sync_info": {"on_update": [],
                                          "on_wait": extra[i:i + 2]},
                        })
                    si["on_wait"] = rest
                out.append(inst)
            blk["instructions"] = out
    return json.dumps(j).encode()


def _install_shim():
    import concourse.bass2jax as b2j
    import concourse.bass_utils as bu
    if getattr(bu, "_wsplit_installed", False):
        return
    orig = bu.compile_bir_kernel

    def patched(bir_json, tmpdir, neff_name="file.neff"):
        return orig(_split_excess_waits(bir_json), tmpdir, neff_name)

    bu.compile_bir_kernel = patched
    bu._wsplit_installed = True
    b2j.compile_bir_kernel = patched


# --------------------------------------------------------------------------
def build_nc():
    nc = bass.Bass()

    xT_d = nc.dram_tensor("xT", [128, 5, R], BF, kind="ExternalInput")
    wih0T_d = nc.dram_tensor("wih0T", [128, 5, 4096], BF, kind="ExternalInput")
    wih1T_d = nc.dram_tensor("wih1T", [128, 9, 4096], BF, kind="ExternalInput")
    whh0T_d = nc.dram_tensor("whh0T", [128, 4, 4096], BF, kind="ExternalInput")
    whh1T_d = nc.dram_tensor("whh1T", [128, 4, 4096], BF, kind="ExternalInput")
    h00_d = nc.dram_tensor("h00", [128, 8, BL], BF, kind="ExternalInput")
    h01_d = nc.dram_tensor("h01", [128, 8, BL], BF, kind="ExternalInput")
    c00_d = nc.dram_tensor("c00", [128, 8, BL], F32, kind="ExternalInput")
    c01_d = nc.dram_tensor("c01", [128, 8, BL], F32, kind="ExternalInput")
    eye_d = nc.dram_tensor("eye", [128, 128], BF, kind="ExternalInput")
    if USE_FP8_FC:
        fcwT_d = nc.dram_tensor("fcwT", [128, 8, V], F8, kind="ExternalInput")
        fcrT_d = nc.dram_tensor("fcrT", [128, 8, V], F8, kind="ExternalInput")
    else:
        fcwT_d = nc.dram_tensor("fcwT", [128, 8, V], BF, kind="ExternalInput")
    out_d = nc.dram_tensor("logitsT", [128, NV, R], BF, kind="ExternalOutput")

    with tile.TileContext(nc) as tc:
        with tc.tile_pool(name="persist", bufs=1) as per:
            eye = per.tile([128, 128], BF)
            nc.sync.dma_start(eye[:], eye_d[:])
            xT = per.tile([128, 5, R], BF)
            nc.sync.dma_start(xT[:], xT_d[:])
            ones_row = xT[:, 4, :]          # [128, R], row0 = ones

            # ---------- recurrence (shared for both layers) ----------
            def recurrence(tag, gihT, whhT, hcT_init_d, c_init_d, dstT,
                           hook=None):
                """gihT: [128, 32, R] bf16; whhT: [128, 4, 4096] bf16;
                dstT: persistent tile [128, 8, R+BL] bf16 (rows 0..BL = h_init).
                """
                with (
                    tc.tile_pool(name=f"r{tag}s", bufs=2) as sp,
                    tc.tile_pool(name=f"r{tag}p", bufs=1, space="PSUM") as pp,
                ):
                    nc.sync.dma_start(dstT[:, :, 0:BL], hcT_init_d[:])
                    cT = sp.tile([128, 8, BL], F32, tag="cT")
                    nc.sync.dma_start(cT[:], c_init_d[:])
                    ORDER = (1, 0, 2, 3)     # f, i, g, o
                    for s in range(S):
                        ps = {}
                        for t in ORDER:
                            p = pp.tile([128, 8, BL], F32, tag=f"ps{t}")
                            # inject gih for both cells of gate-type t
                            nc.tensor.matmul(
                                p[:], eye[:],
                                gihT[:, 8 * t:8 * t + 8, BL * s:BL * s + BL],
                                start=True, stop=False)
                            for c in range(2):
                                for j in range(4):
                                    gcol = (t * 8 + c * 4 + j) * 128
                                    for k in range(4):
                                        nc.tensor.matmul(
                                            p[:, 4 * c + j, :],
                                            whhT[:, k, gcol:gcol + 128],
                                            dstT[:, 4 * c + k,
                                                 BL * s:BL * s + BL],
                                            start=False,
                                            stop=(k == 3),
                                        )
                            ps[t] = p
                        gs = {}
                        for t in ORDER:
                            g = sp.tile([128, 8, BL], F32, tag=f"g{t}")
                            nc.scalar.activation(
                                g[:], ps[t][:],
                                AF.Tanh if t == 2 else AF.Sigmoid)
                            gs[t] = g
                        t1 = sp.tile([128, 8, BL], F32, tag="t1")
                        nc.vector.tensor_mul(t1[:], gs[1][:], cT[:])
                        t2 = sp.tile([128, 8, BL], F32, tag="t2")
                        nc.vector.tensor_mul(t2[:], gs[0][:], gs[2][:])
                        nc.vector.tensor_add(cT[:], t1[:], t2[:])
                        tc2 = sp.tile([128, 8, BL], F32, tag="tc2")
                        nc.scalar.activation(tc2[:], cT[:], AF.Tanh)
                        nc.vector.tensor_mul(
                            dstT[:, :, BL * (s + 1):BL * (s + 2)],
                            gs[3][:], tc2[:])
                        if hook is not None:
                            hook(s)

            # ---------- gih precompute (stationary wihT, moving rows) ----
            def gih_phase(tag, wihT_fn, kcs, rhs_fn, gihT):
                with (
                    tc.tile_pool(name=f"g{tag}p", bufs=4, space="PSUM") as pp,
                ):
                    for G in range(32):
                        wihT, co = wihT_fn(G)
                        p = pp.tile([128, R], F32, tag="ps")
                        for i, kc in enumerate(kcs):
                            nc.tensor.matmul(
                                p[:],
                                wihT[:, kc, 128 * co:128 * (co + 1)],
                                rhs_fn(kc),
                                start=(i == 0), stop=(i == len(kcs) - 1))
                        if G % 2 == 0:
                            nc.vector.tensor_copy(gihT[:, G, :], p[:])
                        else:
                            nc.scalar.activation(gihT[:, G, :], p[:], AF.Copy)

            # ---------- layer 0 (+ interleaved gih1 blocks) ----------
            x1T = per.tile([128, 8, R + BL], BF)
            outsT = per.tile([128, 8, R + BL], BF)
            gih1T = per.tile([128, 32, R], BF)
            whh1T = per.tile([128, 4, 4096], BF)
            with (
                tc.tile_pool(name="w1", bufs=2) as w1p,
                tc.tile_pool(name="g1pp", bufs=2, space="PSUM") as g1pp,
            ):
                def gih1_block(b):       # steps 16b..16b+15
                    r0 = 128 * b
                    w1t = None
                    for G in range(32):
                        if G % 4 == 0:   # stream wih1 512-col chunks
                            ci = G // 4
                            w1t = w1p.tile([128, 9, 512], BF, tag="w1c")
                            nc.sync.dma_start(
                                w1t[:],
                                wih1T_d[:, :, 512 * ci:512 * (ci + 1)])
                        p = g1pp.tile([128, 128], F32, tag="g1ps")
                        for kc in range(9):
                            rhs = (x1T[:, kc, BL + r0:BL + r0 + 128]
                                   if kc < 8 else xT[:, 4, r0:r0 + 128])
                            nc.tensor.matmul(
                                p[:],
                                w1t[:, kc,
                                    128 * (G % 4):128 * (G % 4 + 1)],
                                rhs, start=(kc == 0), stop=(kc == 8))
                        if G % 2 == 0:
                            nc.vector.tensor_copy(
                                gih1T[:, G, r0:r0 + 128], p[:])
                        else:
                            nc.scalar.activation(
                                gih1T[:, G, r0:r0 + 128], p[:], AF.Copy)

                with tc.tile_pool(name="l0", bufs=1) as l0p:
                    gih0T = l0p.tile([128, 32, R], BF)
                    whh0T = l0p.tile([128, 4, 4096], BF)
                    with tc.tile_pool(name="w0", bufs=2) as w0p:
                        w0state = {}

                        def wih0_for(G):
                            ci = G // 4
                            if ci not in w0state:
                                t = w0p.tile([128, 5, 512], BF, tag="w0c")
                                nc.sync.dma_start(
                                    t[:],
                                    wih0T_d[:, :, 512 * ci:512 * (ci + 1)])
                                w0state[ci] = t
                            return w0state[ci], G % 4

                        gih_phase("0", wih0_for,
                                  range(5), lambda kc: xT[:, kc, :], gih0T)
                    nc.sync.dma_start(whh0T[:], whh0T_d[:])
                    nc.sync.dma_start(whh1T[:], whh1T_d[:])

                    def l0_hook(s):
                        if s % 16 == 15 and s < 63:
                            gih1_block(s // 16)

                    recurrence("0", gih0T, whh0T, h00_d, c00_d, x1T,
                               hook=l0_hook)
                gih1_block(3)

            # ---------- layer 1 + FC ----------
            # FC pools open before L1 recurrence so fcw prefetch overlaps
            VG = 10                  # vocab chunks per stream group
            NG = NV // VG            # 25 stream groups
            OG = 5                   # vocab chunks per output DMA tile
            fdt = F8 if USE_FP8_FC else BF
            RSCALE = 256.0
            with tc.tile_pool(name="fcw", bufs=3) as fwp:
                def load_group(g):
                    sl = slice(VG * 128 * g, VG * 128 * (g + 1))
                    fcw = fwp.tile([128, 8, VG * 128], fdt, tag="fcw")
                    nc.sync.dma_start(fcw[:], fcwT_d[:, :, sl])
                    if USE_FP8_FC:
                        fcr = fwp.tile([128, 8, VG * 128], F8, tag="fcr")
                        nc.sync.dma_start(fcr[:], fcrT_d[:, :, sl])
                        return fcw, fcr
                    return fcw, None

                fcws = [load_group(g) for g in range(2)]
                recurrence("1", gih1T, whh1T, h01_d, c01_d, outsT)

                fstk = contextlib.ExitStack()
                fop = fstk.enter_context(tc.tile_pool(name="fco", bufs=2))
                fpp = fstk.enter_context(
                    tc.tile_pool(name="fcp", bufs=4, space="PSUM"))
                fxp = fstk.enter_context(tc.tile_pool(name="fcx", bufs=1))
                if USE_FP8_FC:
                    outsT8 = fxp.tile([128, 8, R], F8)
                    orT8 = fxp.tile([128, 8, R], F8)
                    for kc in range(8):
                        if kc % 2 == 0:
                            nc.vector.tensor_copy(
                                outsT8[:, kc, :], outsT[:, kc, BL:R + BL])
                        else:
                            nc.scalar.activation(
                                outsT8[:, kc, :],
                                outsT[:, kc, BL:R + BL], AF.Copy)
                    for kc in range(8):
                        tmp = fxp.tile([128, R], F32, tag=f"or{kc % 2}")
                        nc.vector.tensor_sub(
                            tmp[:], outsT[:, kc, BL:R + BL], outsT8[:, kc, :])
                        nc.scalar.activation(orT8[:, kc, :], tmp[:], AF.Copy,
                                             scale=RSCALE)
                for g in range(NG):
                    fcw, fcr = fcws[g] if g < 2 else load_group(g)
                    for og in range(VG // OG):
                        ot = fop.tile([128, OG, R], BF, tag="ot")
                        for oi in range(OG):
                            n = og * OG + oi
                            csl = slice(128 * n, 128 * (n + 1))
                            if USE_FP8_FC:
                                pm = fpp.tile([128, R], F32, tag="fpsm")
                                pr = fpp.tile([128, R], F32, tag="fpsr")
                                DR = mybir.MatmulPerfMode.DoubleRow
                                for dk in range(4):
                                    ksl = slice(2 * dk, 2 * dk + 2)
                                    nc.tensor.matmul(
                                        pm[:], fcw[:, ksl, csl],
                                        outsT8[:, ksl, :],
                                        start=(dk == 0), stop=(dk == 3),
                                        perf_mode=DR)
                                for dk in range(4):
                                    ksl = slice(2 * dk, 2 * dk + 2)
                                    nc.tensor.matmul(
                                        pr[:], fcr[:, ksl, csl],
                                        outsT8[:, ksl, :],
                                        start=(dk == 0), stop=False,
                                        perf_mode=DR)
                                for dk in range(4):
                                    ksl = slice(2 * dk, 2 * dk + 2)
                                    nc.tensor.matmul(
                                        pr[:], fcw[:, ksl, csl],
                                        orT8[:, ksl, :],
                                        start=False, stop=(dk == 3),
                                        perf_mode=DR)
                                rt = fop.tile([128, R], F32, tag="rt")
                                nc.scalar.activation(rt[:], pr[:], AF.Copy,
                                                     scale=1.0 / RSCALE)
                                nc.vector.tensor_add(ot[:, oi, :], pm[:],
                                                     rt[:])
                            else:
                                p = fpp.tile([128, R], F32, tag="fpsm")
                                for kc in range(8):
                                    nc.tensor.matmul(
                                        p[:], fcw[:, kc, csl],
                                        outsT[:, kc, BL:R + BL],
                                        start=(kc == 0), stop=(kc == 7))
                                if oi % 2 == 0:
                                    nc.vector.tensor_copy(ot[:, oi, :], p[:])
                                else:
                                    nc.scalar.activation(ot[:, oi, :], p[:],
                                                         AF.Copy)
                        nv0 = g * VG + og * OG
                        nc.gpsimd.dma_start(out_d[:, nv0:nv0 + OG, :],
                                            ot[:])
                fstk.close()
    return nc


_NC_CACHE = None


def _pack_shared(w_ih_l0, w_hh_l0, b_ih_l0, b_hh_l0, w_ih_l1, w_hh_l1,
                 b_ih_l1, b_hh_l1, fc_w):
    """Weight packing identical across cores."""
    def gate_perm():
        # column permutation: new chunk t*8+c*4+j <- old gate block
        # old gate index within cat(cell0, cell1): c*2048 + t*512 + j*128
        perm = np.empty(4096, np.int64)
        pos = 0
        for t in range(4):
            for c in range(2):
                for j in range(4):
                    src = c * 2048 + t * 512 + j * 128
                    perm[pos:pos + 128] = np.arange(src, src + 128)
                    pos += 128
        return perm

    PERM = gate_perm()

    def pack_wih(w_cat, b_cat, kchunks):
        # w_cat: (4096 gates, K) fp32; returns [128, kchunks, 4096] bf16
        gdim, kk = w_cat.shape
        kc_data = kk // 128
        out = np.zeros((128, kchunks, 4096), BF16)
        wp = w_cat[PERM]
        for kc in range(kc_data):
            out[:, kc, :] = wp[:, 128 * kc:128 * (kc + 1)].T.astype(BF16)
        out[0, kc_data, :] = b_cat[PERM].astype(BF16)
        return out

    b0 = (b_ih_l0 + b_hh_l0).astype(np.float32)
    b1 = (b_ih_l1 + b_hh_l1).astype(np.float32)
    wih0 = np.vstack([w_ih_l0[0], w_ih_l0[1]]).astype(np.float32)
    wih1 = np.vstack([w_ih_l1[0], w_ih_l1[1]]).astype(np.float32)
    whh0 = np.vstack([w_hh_l0[0], w_hh_l0[1]]).astype(np.float32)
    whh1 = np.vstack([w_hh_l1[0], w_hh_l1[1]]).astype(np.float32)
    b0c = np.concatenate([b0[0], b0[1]])
    b1c = np.concatenate([b1[0], b1[1]])

    wih0T = pack_wih(wih0, b0c, 5)
    wih1T = pack_wih(wih1, b1c, 9)

    def pack_whh(w_cat):
        out = np.zeros((128, 4, 4096), BF16)
        wp = w_cat[PERM]
        for k in range(4):
            out[:, k, :] = wp[:, 128 * k:128 * (k + 1)].T.astype(BF16)
        return out

    whh0T = pack_whh(whh0)
    whh1T = pack_whh(whh1)

    fc = np.asarray(fc_w, np.float32)        # (V, 1024)
    fcT = np.ascontiguousarray(fc.reshape(V, 8, 128).transpose(2, 1, 0))
    if USE_FP8_FC:
        fcwT = fcT.astype(FP8)
        fcrT = ((fcT - fcwT.astype(np.float32)) * 256.0).astype(FP8)
    else:
        fcwT = fcT.astype(BF16)
        fcrT = None

    eye = np.eye(128, dtype=np.float32).astype(BF16)
    return wih0T, wih1T, whh0T, whh1T, fcwT, fcrT, eye


def _pack_inputs(hidden_state, cell_state, Y, emb, w_ih_l0, w_hh_l0, b_ih_l0,
                 b_hh_l0, w_ih_l1, w_hh_l1, b_ih_l1, b_hh_l1, fc_w, fc_b):
    wih0T, wih1T, whh0T, whh1T, fcwT, fcrT, eye = _pack_shared(
        w_ih_l0, w_hh_l0, b_ih_l0, b_hh_l0, w_ih_l1, w_hh_l1,
        b_ih_l1, b_hh_l1, fc_w)

    Y = np.asarray(Y)
    idx_seq = np.concatenate([Y[:, 1:2], Y[:, :-1]], axis=1)   # (B, S)
    emb = np.asarray(emb, np.float32)
    hs = np.asarray(hidden_state, np.float32)   # (4, B, H)
    cs = np.asarray(cell_state, np.float32)

    ins = []
    for core in range(NC):
        bsl = slice(BL * core, BL * (core + 1))
        idx = idx_seq[bsl]                       # (BL, S)
        x = emb[idx.T.reshape(-1)]               # (R, E), r = 8s+bl
        xT = np.zeros((128, 5, R), BF16)
        for kc in range(4):
            xT[:, kc, :] = x[:, 128 * kc:128 * (kc + 1)].T.astype(BF16)
        xT[0, 4, :] = BF16(1.0)

        def pack_state(arr, cells, np_dt):
            # -> [128, 8(c*4+j), BL]
            out = np.zeros((128, 8, BL), np_dt)
            for ci, cell in enumerate(cells):
                a = arr[cell][bsl]               # (BL, H)
                for j in range(4):
                    out[:, 4 * ci + j, :] = \
                        a[:, 128 * j:128 * (j + 1)].T.astype(np_dt)
            return out

        m = {
            "xT": xT,
            "wih0T": wih0T, "wih1T": wih1T,
            "whh0T": whh0T, "whh1T": whh1T,
            "h00": pack_state(hs, (0, 1), BF16),
            "h01": pack_state(hs, (2, 3), BF16),
            "c00": pack_state(cs, (0, 1), np.float32),
            "c01": pack_state(cs, (2, 3), np.float32),
            "eye": eye, "fcwT": fcwT,
        }
        if USE_FP8_FC:
            m["fcrT"] = fcrT
        ins.append(m)
    return ins


def kernel(**inputs):
    global _NC_CACHE
    _install_shim()
    if _NC_CACHE is None:
        _NC_CACHE = build_nc()
    nc = _NC_CACHE
    in_maps = _pack_inputs(**inputs)
    res = run_bass_kernel_spmd(nc, in_maps, list(range(NC)))
    fc_b = np.asarray(inputs["fc_b"], np.float32)
    full = np.empty((B, S, V), np.float32)
    for core in range(NC):
        arr = np.asarray(res.results[core]["logitsT"])    # [128, NV, R] bf16
        lg = arr.transpose(2, 1, 0).reshape(R, V).astype(np.float32)
        # rows r = 8s + bl -> (s, bl)
        full[BL * core:BL * (core + 1)] = \
            lg.reshape(S, BL, V).transpose(1, 0, 2)
    full += fc_b[None, None, :]
    return full.reshape(B * S, V)


# revision 28
# speedup vs baseline: 3.7557x; 1.1186x over previous
"""Trainium2 Bass kernel for nn_Decoder (2-layer bidirectional-structure LSTM
decoder + vocab projection), SPMD across 8 NeuronCores.

Strategy (v2): pure batch sharding, 8 batch rows per core, zero collectives.
All state is kept TRANSPOSED ([dim on partitions, batch on free]) so the
recurrence needs no per-step transposes:

  - gih0T = wih0^T-stationary matmuls over xT  (gates on partitions, rows on
    free), kept in SBUF.
  - L0 recurrence: per step, per gate-type t: one eye-matmul injects the gih
    slice for both cells into PSUM [128, 8(c,j), 8], then 2c*4j*4k hh-matmuls
    (N=8) accumulate h @ whh^T.  Gates -> sigmoid/tanh on ACT (one op per
    gate type, both cells merged), c/h updates on DVE.  h lands directly in
    x1T (transposed) which feeds both the next step's matmuls and gih1.
  - gih1T from x1T, L1 recurrence likewise -> outsT.
  - FC: fp8e4m3 DoubleRow matmuls (fc_w^T stationary, outsT8 moving),
    full 32000 vocab per core, logitsT written back vocab-chunk-major bf16.
  - Host: embedding gather, weight transposes/packing (shared across cores),
    final (vocab,row)->(row,vocab) reorder, fc_b add.
"""
import contextlib
import json
import os
import sys

sys.path.insert(0, "/opt/trn_rl_repo")

import ml_dtypes
import numpy as np

import concourse.bass as bass
import concourse.tile as tile
from concourse import mybir
from concourse.bass_utils import run_bass_kernel_spmd

BF16 = ml_dtypes.bfloat16
FP8 = ml_dtypes.float8_e4m3
V, E, H, B, S = 32000, 512, 512, 64, 64
NC = 8
BL = B // NC              # 8 batch rows per core
R = S * BL                # 512 rows per core, r = 8*s + bl
NV = V // 128             # 250 vocab chunks

F32 = mybir.dt.float32
BF = mybir.dt.bfloat16
F8 = mybir.dt.float8e4
AF = mybir.ActivationFunctionType

USE_FP8_FC = os.environ.get("BASS_FC_BF16", "") == ""

# gih / whh gate-chunk order: chunk = t*8 + c*4 + j
#   t: gate type in PyTorch order (i=0, f=1, g=2, o=3)
#   c: cell within layer (0=fwd, 1=bwd), j: h-dim chunk (128 wide)
# elementwise tiles are [128, 8(c,j slot=4c+j), 8(batch)]


# --------------------------------------------------------------------------
# walrus workaround: this build allows at most 2 sem waits per instruction.
def _split_excess_waits(bir_json):
    j = json.loads(bir_json)
    n = 0
    for fn in j.get("functions", []):
        for blk in fn.get("blocks", []):
            out = []
            for inst in blk.get("instructions", []):
                si = inst.get("sync_info")
                ow = (si or {}).get("on_wait") or []
                keep = 2 if inst.get("opcode") == "EventSemaphore" else 1
                if len(ow) > keep:
                    extra, rest = ow[:-keep], ow[-keep:]
                    for i in range(0, len(extra), 2):
                        n += 1
                        out.append({
                            "debug": inst.get("debug", 0),
                            "engine": inst["engine"],
                            "ins": [], "outs": [],
                            "name": f"WSPLIT-{n}",
                            "opcode": "EventSemaphore",
                            "sync_info": {"on_update": [],
                                          "on_wait": extra[i:i + 2]},
                        })
                    si["on_wait"] = rest
                out.append(inst)
            blk["instructions"] = out
    return json.dumps(j).encode()


def _install_shim():
    import concourse.bass2jax as b2j
    import concourse.bass_utils as bu
    if getattr(bu, "_wsplit_installed", False):
        return
    orig = bu.compile_bir_kernel

    def patched(bir_json, tmpdir, neff_name="file.neff"):
        return orig(_split_excess_waits(bir_json), tmpdir, neff_name)

    bu.compile_bir_kernel = patched
    bu._wsplit_installed = True
    b2j.compile_bir_kernel = patched


# --------------------------------------------------------------------------
def build_nc():
    nc = bass.Bass()

    xT_d = nc.dram_tensor("xT", [128, 5, R], BF, kind="ExternalInput")
    wih0T_d = nc.dram_tensor("wih0T", [128, 5, 4096], BF, kind="ExternalInput")
    wih1T_d = nc.dram_tensor("wih1T", [128, 9, 4096], BF, kind="ExternalInput")
    whh0T_d = nc.dram_tensor("whh0T", [128, 4, 4096], BF, kind="ExternalInput")
    whh1T_d = nc.dram_tensor("whh1T", [128, 4, 4096], BF, kind="ExternalInput")
    h00_d = nc.dram_tensor("h00", [128, 8, BL], BF, kind="ExternalInput")
    h01_d = nc.dram_tensor("h01", [128, 8, BL], BF, kind="ExternalInput")
    c00_d = nc.dram_tensor("c00", [128, 8, BL], F32, kind="ExternalInput")
    c01_d = nc.dram_tensor("c01", [128, 8, BL], F32, kind="ExternalInput")
    eye_d = nc.dram_tensor("eye", [128, 128], BF, kind="ExternalInput")
    if USE_FP8_FC:
        fcwT_d = nc.dram_tensor("fcwT", [128, 8, V], F8, kind="ExternalInput")
        fcrT_d = nc.dram_tensor("fcrT", [128, 8, V], F8, kind="ExternalInput")
    else:
        fcwT_d = nc.dram_tensor("fcwT", [128, 8, V], BF, kind="ExternalInput")
    out_d = nc.dram_tensor("logitsT", [128, NV, R], BF, kind="ExternalOutput")

    with tile.TileContext(nc) as tc:
        with tc.tile_pool(name="persist", bufs=1) as per:
            eye = per.tile([128, 128], BF)
            nc.sync.dma_start(eye[:], eye_d[:])
            xT = per.tile([128, 5, R], BF)
            nc.sync.dma_start(xT[:], xT_d[:])
            ones_row = xT[:, 4, :]          # [128, R], row0 = ones
            x1T = per.tile([128, 8, R + BL], BF)
            outsT = per.tile([128, 8, R + BL], BF)

            # ---------- gih precompute (stationary wihT, moving rows) ----
            def gih_phase(tag, wihT_fn, kcs, rhs_fn, gihT):
                with (
                    tc.tile_pool(name=f"g{tag}p", bufs=4, space="PSUM") as pp,
                ):
                    for G in range(32):
                        wihT, co = wihT_fn(G)
                        p = pp.tile([128, R], F32, tag="ps")
                        for i, kc in enumerate(kcs):
                            nc.tensor.matmul(
                                p[:],
                                wihT[:, kc, 128 * co:128 * (co + 1)],
                                rhs_fn(kc),
                                start=(i == 0), stop=(i == len(kcs) - 1))
                        if G % 2 == 0:
                            nc.vector.tensor_copy(gihT[:, G, :], p[:])
                        else:
                            nc.scalar.activation(gihT[:, G, :], p[:], AF.Copy)

            VG = 10                  # vocab chunks per FC stream group
            NG = NV // VG
            OG = 5                   # vocab chunks per output DMA tile
            fdt = F8 if USE_FP8_FC else BF
            RSCALE = 256.0

            with tc.tile_pool(name="fcw", bufs=2) as fwp:
                def load_group(g):
                    sl = slice(VG * 128 * g, VG * 128 * (g + 1))
                    fcw = fwp.tile([128, 8, VG * 128], fdt, tag="fcw")
                    nc.sync.dma_start(fcw[:], fcwT_d[:, :, sl])
                    if USE_FP8_FC:
                        fcr = fwp.tile([128, 8, VG * 128], F8, tag="fcr")
                        nc.sync.dma_start(fcr[:], fcrT_d[:, :, sl])
                        return fcw, fcr
                    return fcw, None

                with (
                    tc.tile_pool(name="lw", bufs=1) as lwp,
                    tc.tile_pool(name="w1", bufs=2) as w1p,
                ):
                    gih0T = lwp.tile([128, 32, R], BF)
                    gih1T = lwp.tile([128, 32, R], BF)
                    whh0T = lwp.tile([128, 4, 4096], BF)
                    whh1T = lwp.tile([128, 4, 4096], BF)
                    with tc.tile_pool(name="w0", bufs=2) as w0p:
                        w0state = {}

                        def wih0_for(G):
                            ci = G // 2
                            if ci not in w0state:
                                t = w0p.tile([128, 5, 256], BF, tag="w0c")
                                nc.sync.dma_start(
                                    t[:],
                                    wih0T_d[:, :, 256 * ci:256 * (ci + 1)])
                                w0state[ci] = t
                            return w0state[ci], G % 2

                        gih_phase("0", wih0_for,
                                  range(5), lambda kc: xT[:, kc, :], gih0T)
                    nc.sync.dma_start(whh0T[:], whh0T_d[:])
                    nc.sync.dma_start(whh1T[:], whh1T_d[:])

                    # ---------- fused L0+L1 recurrence pipeline ----------
                    LAG = 33
                    with (
                        tc.tile_pool(name="rs", bufs=2) as sp,
                        tc.tile_pool(name="rp", bufs=1, space="PSUM") as pp,
                    ):
                        def gih1_pair(u):
                            # supersteps 16..79: block b=(u-16)//16, 2 G-chunks
                            b = (u - 16) // 16
                            pi = (u - 16) % 16
                            r0 = 128 * b
                            w1t = w1p.tile([128, 9, 256], BF, tag="w1c")
                            nc.sync.dma_start(
                                w1t[:],
                                wih1T_d[:, :, 256 * pi:256 * (pi + 1)])
                            for gi in range(2):
                                G = 2 * pi + gi
                                p = pp.tile([128, 128], F32, tag="L0ps3")
                                for kc in range(9):
                                    rhs = (x1T[:, kc, BL + r0:BL + r0 + 128]
                                           if kc < 8
                                           else xT[:, 4, r0:r0 + 128])
                                    nc.tensor.matmul(
                                        p[:],
                                        w1t[:, kc, 128 * gi:128 * (gi + 1)],
                                        rhs, start=(kc == 0), stop=(kc == 8))
                                if G % 2 == 0:
                                    nc.vector.tensor_copy(
                                        gih1T[:, G, r0:r0 + 128], p[:])
                                else:
                                    nc.scalar.activation(
                                        gih1T[:, G, r0:r0 + 128], p[:],
                                        AF.Copy)

                        cT = {}
                        for li, (h_d, c_d, dstT) in enumerate(
                                ((h00_d, c00_d, x1T), (h01_d, c01_d, outsT))):
                            nc.sync.dma_start(dstT[:, :, 0:BL], h_d[:])
                            ct_tile = sp.tile([128, 8, BL], F32,
                                              tag=f"cT{li}")
                            cT[li] = ct_tile
                            nc.sync.dma_start(ct_tile[:], c_d[:])

                        ORDER = (1, 0, 2, 3)     # f, i, g, o

                        def mm_stage(li, gihT, whhT, dstT, s, t, pstore):
                            p = pp.tile([128, 8, BL], F32, tag=f"L{li}ps{t}")
                            nc.tensor.matmul(
                                p[:], eye[:],
                                gihT[:, 8 * t:8 * t + 8, BL * s:BL * s + BL],
                                start=True, stop=False)
                            for c in range(2):
                                for j in range(4):
                                    gcol = (t * 8 + c * 4 + j) * 128
                                    for k in range(4):
                                        nc.tensor.matmul(
                                            p[:, 4 * c + j, :],
                                            whhT[:, k, gcol:gcol + 128],
                                            dstT[:, 4 * c + k,
                                                 BL * s:BL * s + BL],
                                            start=False, stop=(k == 3))
                            pstore[t] = p

                        def fused_step(parts):
                            # parts: list of (li, gihT, whhT, dstT, s)
                            ps = {li: {} for (li, *_r) in parts}
                            gs = {li: {} for (li, *_r) in parts}

                            def act_stage(li, t):
                                g = sp.tile([128, 8, BL], F32,
                                            tag=f"g{li}{t}")
                                nc.scalar.activation(
                                    g[:], ps[li][t][:],
                                    AF.Tanh if t == 2 else AF.Sigmoid)
                                gs[li][t] = g

                            for li, gihT, whhT, dstT, s in parts:
                                mm_stage(li, gihT, whhT, dstT, s, 1, ps[li])
                            for li, *_r in parts:
                                act_stage(li, 1)
                            for li, gihT, whhT, dstT, s in parts:
                                mm_stage(li, gihT, whhT, dstT, s, 0, ps[li])
                            t1 = {}
                            for li, *_r in parts:
                                t1t = sp.tile([128, 8, BL], F32,
                                              tag=f"t1{li}")
                                t1[li] = t1t
                                nc.vector.tensor_mul(t1t[:], gs[li][1][:],
                                                     cT[li][:])
                                act_stage(li, 0)
                            for li, gihT, whhT, dstT, s in parts:
                                mm_stage(li, gihT, whhT, dstT, s, 2, ps[li])
                            for li, *_r in parts:
                                act_stage(li, 2)
                            for li, gihT, whhT, dstT, s in parts:
                                mm_stage(li, gihT, whhT, dstT, s, 3, ps[li])
                            t2 = {}
                            for li, *_r in parts:
                                t2t = sp.tile([128, 8, BL], F32,
                                              tag=f"t2{li}")
                                t2[li] = t2t
                                nc.vector.tensor_mul(t2t[:], gs[li][0][:],
                                                     gs[li][2][:])
                            for li, *_r in parts:
                                nc.vector.tensor_add(cT[li][:], t1[li][:],
                                                     t2[li][:])
                                act_stage(li, 3)
                            tc2 = {}
                            for li, *_r in parts:
                                tct = sp.tile([128, 8, BL], F32,
                                              tag=f"tc{li}")
                                tc2[li] = tct
                                nc.scalar.activation(tct[:], cT[li][:],
                                                     AF.Tanh)
                            for li, gihT, whhT, dstT, s in parts:
                                nc.vector.tensor_mul(
                                    dstT[:, :, BL * (s + 1):BL * (s + 2)],
                                    gs[li][3][:], tc2[li][:])

                        fcws = []
                        for u in range(S + LAG):
                            parts = []
                            if u < S:
                                parts.append((0, gih0T, whh0T, x1T, u))
                            if u >= LAG:
                                parts.append((1, gih1T, whh1T, outsT,
                                              u - LAG))
                            fused_step(parts)
                            if 16 <= u < 80:
                                gih1_pair(u)
                            if u == S + LAG - 3 and not fcws:
                                fcws.append(load_group(0))

                # ---------- FC ----------
                fstk = contextlib.ExitStack()
                fop = fstk.enter_context(tc.tile_pool(name="fco", bufs=2))
                fpp = fstk.enter_context(
                    tc.tile_pool(name="fcp", bufs=4, space="PSUM"))
                fxp = fstk.enter_context(tc.tile_pool(name="fcx", bufs=1))
                if USE_FP8_FC:
                    outsT8 = fxp.tile([128, 8, R], F8)
                    orT8 = fxp.tile([128, 8, R], F8)
                    for kc in range(8):
                        if kc % 2 == 0:
                            nc.vector.tensor_copy(
                                outsT8[:, kc, :], outsT[:, kc, BL:R + BL])
                        else:
                            nc.scalar.activation(
                                outsT8[:, kc, :],
                                outsT[:, kc, BL:R + BL], AF.Copy)
                    for kc in range(8):
                        tmp = fxp.tile([128, R], F32, tag=f"or{kc % 2}")
                        nc.vector.tensor_sub(
                            tmp[:], outsT[:, kc, BL:R + BL], outsT8[:, kc, :])
                        nc.scalar.activation(orT8[:, kc, :], tmp[:], AF.Copy,
                                             scale=RSCALE)
                for g in range(NG):
                    fcw, fcr = fcws[g] if g < len(fcws) else load_group(g)
                    for og in range(VG // OG):
                        ot = fop.tile([128, OG, R], BF, tag="ot")
                        for oi in range(OG):
                            n = og * OG + oi
                            csl = slice(128 * n, 128 * (n + 1))
                            if USE_FP8_FC:
                                pm = fpp.tile([128, R], F32, tag="fpsm")
                                pr = fpp.tile([128, R], F32, tag="fpsr")
                                DR = mybir.MatmulPerfMode.DoubleRow
                                for dk in range(4):
                                    ksl = slice(2 * dk, 2 * dk + 2)
                                    nc.tensor.matmul(
                                        pm[:], fcw[:, ksl, csl],
                                        outsT8[:, ksl, :],
                                        start=(dk == 0), stop=(dk == 3),
                                        perf_mode=DR)
                                for dk in range(4):
                                    ksl = slice(2 * dk, 2 * dk + 2)
                                    nc.tensor.matmul(
                                        pr[:], fcr[:, ksl, csl],
                                        outsT8[:, ksl, :],
                                        start=(dk == 0), stop=False,
                                        perf_mode=DR)
                                for dk in range(4):
                                    ksl = slice(2 * dk, 2 * dk + 2)
                                    nc.tensor.matmul(
                                        pr[:], fcw[:, ksl, csl],
                                        orT8[:, ksl, :],
                                        start=False, stop=(dk == 3),
                                        perf_mode=DR)
                                rt = fop.tile([128, R], F32, tag="rt")
                                nc.scalar.activation(rt[:], pr[:], AF.Copy,
                                                     scale=1.0 / RSCALE)
                                nc.vector.tensor_add(ot[:, oi, :], pm[:],
                                                     rt[:])
                            else:
                                p = fpp.tile([128, R], F32, tag="fpsm")
                                for kc in range(8):
                                    nc.tensor.matmul(
                                        p[:], fcw[:, kc, csl],
                                        outsT[:, kc, BL:R + BL],
                                        start=(kc == 0), stop=(kc == 7))
                                if oi % 2 == 0:
                                    nc.vector.tensor_copy(ot[:, oi, :], p[:])
                                else:
                                    nc.scalar.activation(ot[:, oi, :], p[:],
                                                         AF.Copy)
                        nv0 = g * VG + og * OG
                        nc.gpsimd.dma_start(out_d[:, nv0:nv0 + OG, :],
                                            ot[:])
                fstk.close()
    return nc


_NC_CACHE = None


def _pack_shared(w_ih_l0, w_hh_l0, b_ih_l0, b_hh_l0, w_ih_l1, w_hh_l1,
                 b_ih_l1, b_hh_l1, fc_w):
    """Weight packing identical across cores."""
    def gate_perm():
        # column permutation: new chunk t*8+c*4+j <- old gate block
        # old gate index within cat(cell0, cell1): c*2048 + t*512 + j*128
        perm = np.empty(4096, np.int64)
        pos = 0
        for t in range(4):
            for c in range(2):
                for j in range(4):
                    src = c * 2048 + t * 512 + j * 128
                    perm[pos:pos + 128] = np.arange(src, src + 128)
                    pos += 128
        return perm

    PERM = gate_perm()

    def pack_wih(w_cat, b_cat, kchunks):
        # w_cat: (4096 gates, K) fp32; returns [128, kchunks, 4096] bf16
        gdim, kk = w_cat.shape
        kc_data = kk // 128
        out = np.zeros((128, kchunks, 4096), BF16)
        wp = w_cat[PERM]
        for kc in range(kc_data):
            out[:, kc, :] = wp[:, 128 * kc:128 * (kc + 1)].T.astype(BF16)
        out[0, kc_data, :] = b_cat[PERM].astype(BF16)
        return out

    b0 = (b_ih_l0 + b_hh_l0).astype(np.float32)
    b1 = (b_ih_l1 + b_hh_l1).astype(np.float32)
    wih0 = np.vstack([w_ih_l0[0], w_ih_l0[1]]).astype(np.float32)
    wih1 = np.vstack([w_ih_l1[0], w_ih_l1[1]]).astype(np.float32)
    whh0 = np.vstack([w_hh_l0[0], w_hh_l0[1]]).astype(np.float32)
    whh1 = np.vstack([w_hh_l1[0], w_hh_l1[1]]).astype(np.float32)
    b0c = np.concatenate([b0[0], b0[1]])
    b1c = np.concatenate([b1[0], b1[1]])

    wih0T = pack_wih(wih0, b0c, 5)
    wih1T = pack_wih(wih1, b1c, 9)

    def pack_whh(w_cat):
        out = np.zeros((128, 4, 4096), BF16)
        wp = w_cat[PERM]
        for k in range(4):
            out[:, k, :] = wp[:, 128 * k:128 * (k + 1)].T.astype(BF16)
        return out

    whh0T = pack_whh(whh0)
    whh1T = pack_whh(whh1)

    fc = np.asarray(fc_w, np.float32)        # (V, 1024)
    fcT = np.ascontiguousarray(fc.reshape(V, 8, 128).transpose(2, 1, 0))
    if USE_FP8_FC:
        fcwT = fcT.astype(FP8)
        fcrT = ((fcT - fcwT.astype(np.float32)) * 256.0).astype(FP8)
    else:
        fcwT = fcT.astype(BF16)
        fcrT = None

    eye = np.eye(128, dtype=np.float32).astype(BF16)
    return wih0T, wih1T, whh0T, whh1T, fcwT, fcrT, eye


def _pack_inputs(hidden_state, cell_state, Y, emb, w_ih_l0, w_hh_l0, b_ih_l0,
                 b_hh_l0, w_ih_l1, w_hh_l1, b_ih_l1, b_hh_l1, fc_w, fc_b):
    wih0T, wih1T, whh0T, whh1T, fcwT, fcrT, eye = _pack_shared(
        w_ih_l0, w_hh_l0, b_ih_l0, b_hh_l0, w_ih_l1, w_hh_l1,
        b_ih_l1, b_hh_l1, fc_w)

    Y = np.asarray(Y)
    idx_seq = np.concatenate([Y[:, 1:2], Y[:, :-1]], axis=1)   # (B, S)
    emb = np.asarray(emb, np.float32)
    hs = np.asarray(hidden_state, np.float32)   # (4, B, H)
    cs = np.asarray(cell_state, np.float32)

    ins = []
    for core in range(NC):
        bsl = slice(BL * core, BL * (core + 1))
        idx = idx_seq[bsl]                       # (BL, S)
        x = emb[idx.T.reshape(-1)]               # (R, E), r = 8s+bl
        xT = np.zeros((128, 5, R), BF16)
        for kc in range(4):
            xT[:, kc, :] = x[:, 128 * kc:128 * (kc + 1)].T.astype(BF16)
        xT[0, 4, :] = BF16(1.0)

        def pack_state(arr, cells, np_dt):
            # -> [128, 8(c*4+j), BL]
            out = np.zeros((128, 8, BL), np_dt)
            for ci, cell in enumerate(cells):
                a = arr[cell][bsl]               # (BL, H)
                for j in range(4):
                    out[:, 4 * ci + j, :] = \
                        a[:, 128 * j:128 * (j + 1)].T.astype(np_dt)
            return out

        m = {
            "xT": xT,
            "wih0T": wih0T, "wih1T": wih1T,
            "whh0T": whh0T, "whh1T": whh1T,
            "h00": pack_state(hs, (0, 1), BF16),
            "h01": pack_state(hs, (2, 3), BF16),
            "c00": pack_state(cs, (0, 1), np.float32),
            "c01": pack_state(cs, (2, 3), np.float32),
            "eye": eye, "fcwT": fcwT,
        }
        if USE_FP8_FC:
            m["fcrT"] = fcrT
        ins.append(m)
    return ins


def kernel(**inputs):
    global _NC_CACHE
    _install_shim()
    if _NC_CACHE is None:
        _NC_CACHE = build_nc()
    nc = _NC_CACHE
    in_maps = _pack_inputs(**inputs)
    res = run_bass_kernel_spmd(nc, in_maps, list(range(NC)))
    fc_b = np.asarray(inputs["fc_b"], np.float32)
    full = np.empty((B, S, V), np.float32)
    for core in range(NC):
        arr = np.asarray(res.results[core]["logitsT"])    # [128, NV, R] bf16
        lg = arr.transpose(2, 1, 0).reshape(R, V).astype(np.float32)
        # rows r = 8s + bl -> (s, bl)
        full[BL * core:BL * (core + 1)] = \
            lg.reshape(S, BL, V).transpose(1, 0, 2)
    full += fc_b[None, None, :]
    return full.reshape(B * S, V)


# revision 33
# speedup vs baseline: 3.9234x; 1.0446x over previous
"""Trainium2 Bass kernel for nn_Decoder (2-layer bidirectional-structure LSTM
decoder + vocab projection), SPMD across 8 NeuronCores.

Strategy (v2): pure batch sharding, 8 batch rows per core, zero collectives.
All state is kept TRANSPOSED ([dim on partitions, batch on free]) so the
recurrence needs no per-step transposes:

  - gih0T = wih0^T-stationary matmuls over xT  (gates on partitions, rows on
    free), kept in SBUF.
  - L0 recurrence: per step, per gate-type t: one eye-matmul injects the gih
    slice for both cells into PSUM [128, 8(c,j), 8], then 2c*4j*4k hh-matmuls
    (N=8) accumulate h @ whh^T.  Gates -> sigmoid/tanh on ACT (one op per
    gate type, both cells merged), c/h updates on DVE.  h lands directly in
    x1T (transposed) which feeds both the next step's matmuls and gih1.
  - gih1T from x1T, L1 recurrence likewise -> outsT.
  - FC: fp8e4m3 DoubleRow matmuls (fc_w^T stationary, outsT8 moving),
    full 32000 vocab per core, logitsT written back vocab-chunk-major bf16.
  - Host: embedding gather, weight transposes/packing (shared across cores),
    final (vocab,row)->(row,vocab) reorder, fc_b add.
"""
import contextlib
import json
import os
import sys

sys.path.insert(0, "/opt/trn_rl_repo")

import ml_dtypes
import numpy as np

import concourse.bass as bass
import concourse.tile as tile
from concourse import mybir
from concourse.bass_utils import run_bass_kernel_spmd

BF16 = ml_dtypes.bfloat16
FP8 = ml_dtypes.float8_e4m3
V, E, H, B, S = 32000, 512, 512, 64, 64
NC = 8
BL = B // NC              # 8 batch rows per core
R = S * BL                # 512 rows per core, r = 8*s + bl
NV = V // 128             # 250 vocab chunks

F32 = mybir.dt.float32
BF = mybir.dt.bfloat16
F8 = mybir.dt.float8e4
AF = mybir.ActivationFunctionType

USE_FP8_FC = os.environ.get("BASS_FC_BF16", "") == ""

# gih / whh gate-chunk order: chunk = t*8 + c*4 + j
#   t: gate type in PyTorch order (i=0, f=1, g=2, o=3)
#   c: cell within layer (0=fwd, 1=bwd), j: h-dim chunk (128 wide)
# elementwise tiles are [128, 8(c,j slot=4c+j), 8(batch)]


# --------------------------------------------------------------------------
# walrus workaround: this build allows at most 2 sem waits per instruction.
def _split_excess_waits(bir_json):
    j = json.loads(bir_json)
    n = 0
    for fn in j.get("functions", []):
        for blk in fn.get("blocks", []):
            out = []
            for inst in blk.get("instructions", []):
                si = inst.get("sync_info")
                ow = (si or {}).get("on_wait") or []
                keep = 2 if inst.get("opcode") == "EventSemaphore" else 1
                if len(ow) > keep:
                    extra, rest = ow[:-keep], ow[-keep:]
                    for i in range(0, len(extra), 2):
                        n += 1
                        out.append({
                            "debug": inst.get("debug", 0),
                            "engine": inst["engine"],
                            "ins": [], "outs": [],
                            "name": f"WSPLIT-{n}",
                            "opcode": "EventSemaphore",
                            "sync_info": {"on_update": [],
                                          "on_wait": extra[i:i + 2]},
                        })
                    si["on_wait"] = rest
                out.append(inst)
            blk["instructions"] = out
    return json.dumps(j).encode()


def _install_shim():
    import concourse.bass2jax as b2j
    import concourse.bass_utils as bu
    if getattr(bu, "_wsplit_installed", False):
        return
    orig = bu.compile_bir_kernel

    def patched(bir_json, tmpdir, neff_name="file.neff"):
        return orig(_split_excess_waits(bir_json), tmpdir, neff_name)

    bu.compile_bir_kernel = patched
    bu._wsplit_installed = True
    b2j.compile_bir_kernel = patched


# --------------------------------------------------------------------------
def build_nc():
    nc = bass.Bass()

    xT_d = nc.dram_tensor("xT", [128, 5, R], BF, kind="ExternalInput")
    wih0T_d = nc.dram_tensor("wih0T", [128, 5, 4096], BF, kind="ExternalInput")
    wih1T_d = nc.dram_tensor("wih1T", [128, 9, 4096], BF, kind="ExternalInput")
    whh0T_d = nc.dram_tensor("whh0T", [128, 4, 4096], BF, kind="ExternalInput")
    whh1T_d = nc.dram_tensor("whh1T", [128, 4, 4096], BF, kind="ExternalInput")
    h00_d = nc.dram_tensor("h00", [128, 8, BL], BF, kind="ExternalInput")
    h01_d = nc.dram_tensor("h01", [128, 8, BL], BF, kind="ExternalInput")
    c00_d = nc.dram_tensor("c00", [128, 8, BL], F32, kind="ExternalInput")
    c01_d = nc.dram_tensor("c01", [128, 8, BL], F32, kind="ExternalInput")
    eye_d = nc.dram_tensor("eye", [128, 128], BF, kind="ExternalInput")
    if USE_FP8_FC:
        fcwT_d = nc.dram_tensor("fcwT", [128, 8, V], F8, kind="ExternalInput")
        fcrT_d = nc.dram_tensor("fcrT", [128, 8, V], F8, kind="ExternalInput")
    else:
        fcwT_d = nc.dram_tensor("fcwT", [128, 8, V], BF, kind="ExternalInput")
    out_d = nc.dram_tensor("logitsT", [128, NV, R], BF, kind="ExternalOutput")

    with tile.TileContext(nc) as tc:
        with tc.tile_pool(name="persist", bufs=1) as per:
            eye = per.tile([128, 128], BF)
            nc.sync.dma_start(eye[:], eye_d[:])
            xT = per.tile([128, 5, R], BF)
            nc.sync.dma_start(xT[:], xT_d[:])
            ones_row = xT[:, 4, :]          # [128, R], row0 = ones
            x1T = per.tile([128, 8, R + BL], BF)
            outsT = per.tile([128, 8, R + BL], BF)

            VG = 10                  # vocab chunks per FC stream group
            NG = NV // VG
            OG = 5                   # vocab chunks per output DMA tile
            fdt = F8 if USE_FP8_FC else BF
            RSCALE = 256.0

            with tc.tile_pool(name="fcw", bufs=2) as fwp:
                def load_group(g):
                    sl = slice(VG * 128 * g, VG * 128 * (g + 1))
                    fcw = fwp.tile([128, 8, VG * 128], fdt, tag="fcw")
                    nc.sync.dma_start(fcw[:], fcwT_d[:, :, sl])
                    if USE_FP8_FC:
                        fcr = fwp.tile([128, 8, VG * 128], F8, tag="fcr")
                        nc.sync.dma_start(fcr[:], fcrT_d[:, :, sl])
                        return fcw, fcr
                    return fcw, None

                with (
                    tc.tile_pool(name="lw", bufs=1) as lwp,
                    tc.tile_pool(name="w1", bufs=2) as w1p,
                ):
                    gih0T = lwp.tile([128, 32, R], BF)
                    gih1T = lwp.tile([128, 32, R], BF)
                    whh0T = lwp.tile([128, 4, 4096], BF)
                    whh1T = lwp.tile([128, 4, 4096], BF)
                    w0cm = tc.tile_pool(name="w0", bufs=2)
                    w0p = w0cm.__enter__()
                    g0cm = tc.tile_pool(name="g0pp", bufs=2, space="PSUM")
                    g0pp = g0cm.__enter__()

                    def gih0_gchunk(G, b, w0t, ppool):
                        # gih0T[:, G, 128b:128b+128] from w0 chunk (G//2==pi)
                        r0 = 128 * b
                        p = ppool.tile([128, 128], F32, tag="g0ps")
                        for kc in range(5):
                            nc.tensor.matmul(
                                p[:],
                                w0t[:, kc, 128 * (G % 2):128 * (G % 2 + 1)],
                                xT[:, kc, r0:r0 + 128],
                                start=(kc == 0), stop=(kc == 4))
                        if G % 2 == 0:
                            nc.vector.tensor_copy(
                                gih0T[:, G, r0:r0 + 128], p[:])
                        else:
                            nc.scalar.activation(
                                gih0T[:, G, r0:r0 + 128], p[:], AF.Copy)

                    def load_w0(pi):
                        t = w0p.tile([128, 5, 256], BF, tag="w0c")
                        nc.sync.dma_start(
                            t[:], wih0T_d[:, :, 256 * pi:256 * (pi + 1)])
                        return t

                    # block 0 (rows of steps 0..15) upfront
                    for pi in range(16):
                        w0t = load_w0(pi)
                        gih0_gchunk(2 * pi, 0, w0t, g0pp)
                        gih0_gchunk(2 * pi + 1, 0, w0t, g0pp)

                    def gih0_pair(u, ppool):
                        # supersteps 0..47 produce blocks 1..3
                        b = 1 + u // 16
                        pi = u % 16
                        w0t = load_w0(pi)
                        gih0_gchunk(2 * pi, b, w0t, ppool)
                        gih0_gchunk(2 * pi + 1, b, w0t, ppool)

                    nc.sync.dma_start(whh0T[:], whh0T_d[:])
                    nc.sync.dma_start(whh1T[:], whh1T_d[:])

                    # ---------- fused L0+L1 recurrence pipeline ----------
                    LAG = 33
                    with (
                        tc.tile_pool(name="rs", bufs=1) as sp,
                        tc.tile_pool(name="rp", bufs=1, space="PSUM") as pp,
                    ):
                        def gih1_pair(u):
                            # supersteps 16..79: block b=(u-16)//16, 2 G-chunks
                            b = (u - 16) // 16
                            pi = (u - 16) % 16
                            r0 = 128 * b
                            w1t = w1p.tile([128, 9, 256], BF, tag="w1c")
                            nc.sync.dma_start(
                                w1t[:],
                                wih1T_d[:, :, 256 * pi:256 * (pi + 1)])
                            for gi in range(2):
                                G = 2 * pi + gi
                                p = pp.tile([128, 128], F32, tag="L0ps3")
                                for kc in range(9):
                                    rhs = (x1T[:, kc, BL + r0:BL + r0 + 128]
                                           if kc < 8
                                           else xT[:, 4, r0:r0 + 128])
                                    nc.tensor.matmul(
                                        p[:],
                                        w1t[:, kc, 128 * gi:128 * (gi + 1)],
                                        rhs, start=(kc == 0), stop=(kc == 8))
                                if G % 2 == 0:
                                    nc.vector.tensor_copy(
                                        gih1T[:, G, r0:r0 + 128], p[:])
                                else:
                                    nc.scalar.activation(
                                        gih1T[:, G, r0:r0 + 128], p[:],
                                        AF.Copy)

                        cTm = sp.tile([128, 16, BL], F32, tag="cTm")
                        for li, (h_d, c_d, dstT) in enumerate(
                                ((h00_d, c00_d, x1T), (h01_d, c01_d, outsT))):
                            nc.sync.dma_start(dstT[:, :, 0:BL], h_d[:])
                            nc.sync.dma_start(cTm[:, 8 * li:8 * li + 8, :],
                                              c_d[:])

                        ORDER = (1, 0, 2, 3)     # f, i, g, o

                        def mm_stage(li, gihT, whhT, dstT, s, t, p):
                            h0 = 8 * li
                            nc.tensor.matmul(
                                p[:, h0:h0 + 8, :], eye[:],
                                gihT[:, 8 * t:8 * t + 8, BL * s:BL * s + BL],
                                start=True, stop=False)
                            for c in range(2):
                                for j in range(4):
                                    gcol = (t * 8 + c * 4 + j) * 128
                                    for k in range(4):
                                        nc.tensor.matmul(
                                            p[:, h0 + 4 * c + j, :],
                                            whhT[:, k, gcol:gcol + 128],
                                            dstT[:, 4 * c + k,
                                                 BL * s:BL * s + BL],
                                            start=False, stop=(k == 3))

                        def fused_step(parts):
                            # parts: list of (li, gihT, whhT, dstT, s)
                            lo = 0 if parts[0][0] == 0 else 8
                            hi = 8 + 8 * parts[-1][0]
                            sl = slice(lo, hi)
                            ps = {}
                            gs = {}

                            def act_stage(t):
                                g = sp.tile([128, 16, BL], F32, tag=f"g{t}")
                                nc.scalar.activation(
                                    g[:, sl, :], ps[t][:, sl, :],
                                    AF.Tanh if t == 2 else AF.Sigmoid)
                                gs[t] = g

                            for t in ORDER:
                                p = sp  # placeholder for lint
                                p = pp.tile([128, 16, BL], F32, tag=f"ps{t}")
                                ps[t] = p
                                for li, gihT, whhT, dstT, s in parts:
                                    mm_stage(li, gihT, whhT, dstT, s, t, p)
                                if t == 0:
                                    t1 = sp.tile([128, 16, BL], F32,
                                                 tag="t1")
                                    nc.vector.tensor_mul(
                                        t1[:, sl, :], gs[1][:, sl, :],
                                        cTm[:, sl, :])
                                act_stage(t)
                            t2 = sp.tile([128, 16, BL], F32, tag="t2")
                            nc.vector.tensor_mul(t2[:, sl, :], gs[0][:, sl, :],
                                                 gs[2][:, sl, :])
                            nc.vector.tensor_add(cTm[:, sl, :], t1[:, sl, :],
                                                 t2[:, sl, :])
                            tc2 = sp.tile([128, 16, BL], F32, tag="tc2")
                            nc.scalar.activation(tc2[:, sl, :], cTm[:, sl, :],
                                                 AF.Tanh)
                            for li, gihT, whhT, dstT, s in parts:
                                h0 = 8 * li
                                nc.vector.tensor_mul(
                                    dstT[:, :, BL * (s + 1):BL * (s + 2)],
                                    gs[3][:, h0:h0 + 8, :],
                                    tc2[:, h0:h0 + 8, :])

                        fcws = []
                        for u in range(S + LAG):
                            parts = []
                            if u < S:
                                parts.append((0, gih0T, whh0T, x1T, u))
                            if u >= LAG:
                                parts.append((1, gih1T, whh1T, outsT,
                                              u - LAG))
                            fused_step(parts)
                            if u < 48:
                                gih0_pair(u, g0pp)
                            if 16 <= u < 80:
                                gih1_pair(u)
                            if u == S + LAG - 3 and not fcws:
                                fcws.append(load_group(0))


                    g0cm.__exit__(None, None, None)
                    w0cm.__exit__(None, None, None)
                # ---------- FC ----------
                fstk = contextlib.ExitStack()
                fop = fstk.enter_context(tc.tile_pool(name="fco", bufs=2))
                fpp = fstk.enter_context(
                    tc.tile_pool(name="fcp", bufs=4, space="PSUM"))
                fxp = fstk.enter_context(tc.tile_pool(name="fcx", bufs=1))
                if USE_FP8_FC:
                    outsT8 = fxp.tile([128, 8, R], F8)
                    orT8 = fxp.tile([128, 8, R], F8)
                    for kc in range(8):
                        if kc % 2 == 0:
                            nc.vector.tensor_copy(
                                outsT8[:, kc, :], outsT[:, kc, BL:R + BL])
                        else:
                            nc.scalar.activation(
                                outsT8[:, kc, :],
                                outsT[:, kc, BL:R + BL], AF.Copy)
                    for kc in range(8):
                        tmp = fxp.tile([128, R], F32, tag=f"or{kc % 2}")
                        nc.vector.tensor_sub(
                            tmp[:], outsT[:, kc, BL:R + BL], outsT8[:, kc, :])
                        nc.scalar.activation(orT8[:, kc, :], tmp[:], AF.Copy,
                                             scale=RSCALE)
                for g in range(NG):
                    fcw, fcr = fcws[g] if g < len(fcws) else load_group(g)
                    for og in range(VG // OG):
                        ot = fop.tile([128, OG, R], BF, tag="ot")
                        for oi in range(OG):
                            n = og * OG + oi
                            csl = slice(128 * n, 128 * (n + 1))
                            if USE_FP8_FC:
                                pm = fpp.tile([128, R], F32, tag="fpsm")
                                pr = fpp.tile([128, R], F32, tag="fpsr")
                                DR = mybir.MatmulPerfMode.DoubleRow
                                for dk in range(4):
                                    ksl = slice(2 * dk, 2 * dk + 2)
                                    nc.tensor.matmul(
                                        pm[:], fcw[:, ksl, csl],
                                        outsT8[:, ksl, :],
                                        start=(dk == 0), stop=(dk == 3),
                                        perf_mode=DR)
                                for dk in range(4):
                                    ksl = slice(2 * dk, 2 * dk + 2)
                                    nc.tensor.matmul(
                                        pr[:], fcr[:, ksl, csl],
                                        outsT8[:, ksl, :],
                                        start=(dk == 0), stop=False,
                                        perf_mode=DR)
                                for dk in range(4):
                                    ksl = slice(2 * dk, 2 * dk + 2)
                                    nc.tensor.matmul(
                                        pr[:], fcw[:, ksl, csl],
                                        orT8[:, ksl, :],
                                        start=False, stop=(dk == 3),
                                        perf_mode=DR)
                                rt = fop.tile([128, R], F32, tag="rt")
                                nc.scalar.activation(rt[:], pr[:], AF.Copy,
                                                     scale=1.0 / RSCALE)
                                nc.vector.tensor_add(ot[:, oi, :], pm[:],
                                                     rt[:])
                            else:
                                p = fpp.tile([128, R], F32, tag="fpsm")
                                for kc in range(8):
                                    nc.tensor.matmul(
                                        p[:], fcw[:, kc, csl],
                                        outsT[:, kc, BL:R + BL],
                                        start=(kc == 0), stop=(kc == 7))
                                if oi % 2 == 0:
                                    nc.vector.tensor_copy(ot[:, oi, :], p[:])
                                else:
                                    nc.scalar.activation(ot[:, oi, :], p[:],
                                                         AF.Copy)
                        nv0 = g * VG + og * OG
                        nc.gpsimd.dma_start(out_d[:, nv0:nv0 + OG, :],
                                            ot[:])
                fstk.close()
    return nc


_NC_CACHE = None


def _pack_shared(w_ih_l0, w_hh_l0, b_ih_l0, b_hh_l0, w_ih_l1, w_hh_l1,
                 b_ih_l1, b_hh_l1, fc_w):
    """Weight packing identical across cores."""
    def gate_perm():
        # column permutation: new chunk t*8+c*4+j <- old gate block
        # old gate index within cat(cell0, cell1): c*2048 + t*512 + j*128
        perm = np.empty(4096, np.int64)
        pos = 0
        for t in range(4):
            for c in range(2):
                for j in range(4):
                    src = c * 2048 + t * 512 + j * 128
                    perm[pos:pos + 128] = np.arange(src, src + 128)
                    pos += 128
        return perm

    PERM = gate_perm()

    def pack_wih(w_cat, b_cat, kchunks):
        # w_cat: (4096 gates, K) fp32; returns [128, kchunks, 4096] bf16
        gdim, kk = w_cat.shape
        kc_data = kk // 128
        out = np.zeros((128, kchunks, 4096), BF16)
        wp = w_cat[PERM]
        for kc in range(kc_data):
            out[:, kc, :] = wp[:, 128 * kc:128 * (kc + 1)].T.astype(BF16)
        out[0, kc_data, :] = b_cat[PERM].astype(BF16)
        return out

    b0 = (b_ih_l0 + b_hh_l0).astype(np.float32)
    b1 = (b_ih_l1 + b_hh_l1).astype(np.float32)
    wih0 = np.vstack([w_ih_l0[0], w_ih_l0[1]]).astype(np.float32)
    wih1 = np.vstack([w_ih_l1[0], w_ih_l1[1]]).astype(np.float32)
    whh0 = np.vstack([w_hh_l0[0], w_hh_l0[1]]).astype(np.float32)
    whh1 = np.vstack([w_hh_l1[0], w_hh_l1[1]]).astype(np.float32)
    b0c = np.concatenate([b0[0], b0[1]])
    b1c = np.concatenate([b1[0], b1[1]])

    wih0T = pack_wih(wih0, b0c, 5)
    wih1T = pack_wih(wih1, b1c, 9)

    def pack_whh(w_cat):
        out = np.zeros((128, 4, 4096), BF16)
        wp = w_cat[PERM]
        for k in range(4):
            out[:, k, :] = wp[:, 128 * k:128 * (k + 1)].T.astype(BF16)
        return out

    whh0T = pack_whh(whh0)
    whh1T = pack_whh(whh1)

    fc = np.asarray(fc_w, np.float32)        # (V, 1024)
    fcT = np.ascontiguousarray(fc.reshape(V, 8, 128).transpose(2, 1, 0))
    if USE_FP8_FC:
        fcwT = fcT.astype(FP8)
        fcrT = ((fcT - fcwT.astype(np.float32)) * 256.0).astype(FP8)
    else:
        fcwT = fcT.astype(BF16)
        fcrT = None

    eye = np.eye(128, dtype=np.float32).astype(BF16)
    return wih0T, wih1T, whh0T, whh1T, fcwT, fcrT, eye


def _pack_inputs(hidden_state, cell_state, Y, emb, w_ih_l0, w_hh_l0, b_ih_l0,
                 b_hh_l0, w_ih_l1, w_hh_l1, b_ih_l1, b_hh_l1, fc_w, fc_b):
    wih0T, wih1T, whh0T, whh1T, fcwT, fcrT, eye = _pack_shared(
        w_ih_l0, w_hh_l0, b_ih_l0, b_hh_l0, w_ih_l1, w_hh_l1,
        b_ih_l1, b_hh_l1, fc_w)

    Y = np.asarray(Y)
    idx_seq = np.concatenate([Y[:, 1:2], Y[:, :-1]], axis=1)   # (B, S)
    emb = np.asarray(emb, np.float32)
    hs = np.asarray(hidden_state, np.float32)   # (4, B, H)
    cs = np.asarray(cell_state, np.float32)

    ins = []
    for core in range(NC):
        bsl = slice(BL * core, BL * (core + 1))
        idx = idx_seq[bsl]                       # (BL, S)
        x = emb[idx.T.reshape(-1)]               # (R, E), r = 8s+bl
        xT = np.zeros((128, 5, R), BF16)
        for kc in range(4):
            xT[:, kc, :] = x[:, 128 * kc:128 * (kc + 1)].T.astype(BF16)
        xT[0, 4, :] = BF16(1.0)

        def pack_state(arr, cells, np_dt):
            # -> [128, 8(c*4+j), BL]
            out = np.zeros((128, 8, BL), np_dt)
            for ci, cell in enumerate(cells):
                a = arr[cell][bsl]               # (BL, H)
                for j in range(4):
                    out[:, 4 * ci + j, :] = \
                        a[:, 128 * j:128 * (j + 1)].T.astype(np_dt)
            return out

        m = {
            "xT": xT,
            "wih0T": wih0T, "wih1T": wih1T,
            "whh0T": whh0T, "whh1T": whh1T,
            "h00": pack_state(hs, (0, 1), BF16),
            "h01": pack_state(hs, (2, 3), BF16),
            "c00": pack_state(cs, (0, 1), np.float32),
            "c01": pack_state(cs, (2, 3), np.float32),
            "eye": eye, "fcwT": fcwT,
        }
        if USE_FP8_FC:
            m["fcrT"] = fcrT
        ins.append(m)
    return ins


def kernel(**inputs):
    global _NC_CACHE
    _install_shim()
    if _NC_CACHE is None:
        _NC_CACHE = build_nc()
    nc = _NC_CACHE
    in_maps = _pack_inputs(**inputs)
    res = run_bass_kernel_spmd(nc, in_maps, list(range(NC)))
    fc_b = np.asarray(inputs["fc_b"], np.float32)
    full = np.empty((B, S, V), np.float32)
    for core in range(NC):
        arr = np.asarray(res.results[core]["logitsT"])    # [128, NV, R] bf16
        lg = arr.transpose(2, 1, 0).reshape(R, V).astype(np.float32)
        # rows r = 8s + bl -> (s, bl)
        full[BL * core:BL * (core + 1)] = \
            lg.reshape(S, BL, V).transpose(1, 0, 2)
    full += fc_b[None, None, :]
    return full.reshape(B * S, V)
